# revision 5
# baseline (speedup 1.0000x reference)
"""Causal self-attention (B=4, T=2048, C=1024, H=16) on 8 Trainium2 NeuronCores.

Sharding: data-parallel on batch x tensor-parallel on heads.
  core c -> batch b = c//2, head half hf = c%2 (8 heads of 16).
Each core: QKV projection for its heads (column-parallel W_attn),
flash-style causal attention fully on-chip, row-parallel W_proj
producing a partial [T, C] output; pairs of partials are summed at
unshard time.

All matmuls run in fp32r (TRN2 PE mode: fp32 storage, ~11-bit mantissa
products, bf16-rate throughput at moving-dim >= 256).

Layout trick: host feeds x^T per batch ([C, T]) so QKV projections need
no on-device transposes; attention runs in "scores-transposed" [kv, q]
layout; the softmax denominator is obtained free via a ones-column
appended to V; normalization uses a K=1 PE replication matmul.
"""
import numpy as np
from contextlib import ExitStack

import concourse.bacc as bacc
import concourse.tile as tile
from concourse import mybir
from concourse.bass_utils import run_bass_kernel_spmd

dt = mybir.dt

B, T, C, H = 4, 2048, 1024, 16
D = C // H          # 64
N_CORES = 8
HPC = H // 2        # heads per core = 8
HPR = 4             # heads per round
ROUNDS = HPC // HPR # 2
CT = C // 128       # 8 c-tiles (contraction for projections)
NQT = T // 512      # 4 q-tiles per head
NTT = T // 512      # 4 t-tiles for K/Q projection
NST = T // 128      # 16 s-tiles for V projection

_compiled = None    # (nc, )
last_exec_time_ns = None


def _build():
    nc = bacc.Bacc("TRN2", target_bir_lowering=False, debug=False,
                   num_devices=N_CORES)

    xT = nc.dram_tensor("xT", [C, T], dt.float32r, kind="ExternalInput").ap()
    Wa = nc.dram_tensor("Wa", [C, 3 * 512], dt.float32r, kind="ExternalInput").ap()
    Wp = nc.dram_tensor("Wp", [512, C], dt.float32r, kind="ExternalInput").ap()
    Msk = nc.dram_tensor("Msk", [128, 128], dt.float32r, kind="ExternalInput").ap()
    Out = nc.dram_tensor("outp", [T, C], dt.float32, kind="ExternalOutput").ap()

    with tile.TileContext(nc) as tc, ExitStack() as ctx:
        consts = ctx.enter_context(tc.tile_pool(name="consts", bufs=1))
        kpool = ctx.enter_context(tc.tile_pool(name="kpool", bufs=1))
        qpool = ctx.enter_context(tc.tile_pool(name="qpool", bufs=1))
        vpool = ctx.enter_context(tc.tile_pool(name="vpool", bufs=1))
        apool = ctx.enter_context(tc.tile_pool(name="apool", bufs=1))
        wkq = ctx.enter_context(tc.tile_pool(name="wkq", bufs=8))
        wvp = ctx.enter_context(tc.tile_pool(name="wvp", bufs=8))
        wpp = ctx.enter_context(tc.tile_pool(name="wpp", bufs=1))
        wei = ctx.enter_context(tc.tile_pool(name="wei", bufs=4))
        small = ctx.enter_context(tc.tile_pool(name="small", bufs=4))
        outsb = ctx.enter_context(tc.tile_pool(name="outsb", bufs=4))
        ps_proj = ctx.enter_context(tc.tile_pool(name="ps_proj", bufs=2, space="PSUM"))
        ps_sc = ctx.enter_context(tc.tile_pool(name="ps_sc", bufs=2, space="PSUM"))
        ps_pv = ctx.enter_context(tc.tile_pool(name="ps_pv", bufs=2, space="PSUM"))
        ps_misc = ctx.enter_context(tc.tile_pool(name="ps_misc", bufs=2, space="PSUM"))

        # ---- constants & x^T ----
        xt = consts.tile([128, CT, T], dt.float32r)
        for a in range(CT):
            nc.sync.dma_start(xt[:, a, :], xT[a * 128:(a + 1) * 128, :])
        mask = consts.tile([128, 128], dt.float32r)
        nc.sync.dma_start(mask, Msk)
        ones_f = consts.tile([1, 128], dt.float32)
        nc.vector.memset(ones_f, 1.0)
        ones_r = consts.tile([1, 128], dt.float32r)
        nc.vector.tensor_copy(ones_r, ones_f)
        ones64 = consts.tile([128, NST * HPR], dt.float32)
        nc.vector.memset(ones64, 1.0)

        # attnT: [C_local=512 ch -> 4 part-tiles, T]
        attnT = apool.tile([128, HPC // 2, T], dt.float32r)

        for r in range(ROUNDS):
            # ---- K^T projection: kT[ch, t] for 4 heads (256 ch) ----
            kT = kpool.tile([128, 2, T], dt.float32r)
            qT = qpool.tile([128, 2, T], dt.float32r)
            for kt in range(2):
                col0 = 512 + r * 256 + kt * 128  # K block at cols [512, 1024)
                wk = [wkq.tile([128, 128], dt.float32r, tag="wk", name=f"wk_{r}_{kt}_{a}")
                      for a in range(CT)]
                for a in range(CT):
                    nc.sync.dma_start(wk[a], Wa[a * 128:(a + 1) * 128, col0:col0 + 128])
                for tt in range(NTT):
                    pk = ps_proj.tile([128, 512], dt.float32, tag="proj")
                    for a in range(CT):
                        nc.tensor.matmul(pk, wk[a], xt[:, a, tt * 512:(tt + 1) * 512],
                                         start=(a == 0), stop=(a == CT - 1))
                    nc.vector.tensor_copy(kT[:, kt, tt * 512:(tt + 1) * 512], pk)
                # ---- Q^T projection (cols [0, 512)) ----
                col0 = r * 256 + kt * 128
                wq = [wkq.tile([128, 128], dt.float32r, tag="wk", name=f"wq_{r}_{kt}_{a}")
                      for a in range(CT)]
                for a in range(CT):
                    nc.sync.dma_start(wq[a], Wa[a * 128:(a + 1) * 128, col0:col0 + 128])
                for tt in range(NTT):
                    pq = ps_proj.tile([128, 512], dt.float32, tag="proj")
                    for a in range(CT):
                        nc.tensor.matmul(pq, wq[a], xt[:, a, tt * 512:(tt + 1) * 512],
                                         start=(a == 0), stop=(a == CT - 1))
                    nc.vector.tensor_copy(qT[:, kt, tt * 512:(tt + 1) * 512], pq)

            # ---- V projection, row-major with a ones column: v[s, lh, 64] = 1
            v = vpool.tile([128, NST, HPR, 65], dt.float32r)
            col0 = 2 * 512 + r * 256  # V block at cols [1024, 1536)
            wv = [wvp.tile([128, 256], dt.float32r, tag="wv", name=f"wv_{r}_{a}")
                  for a in range(CT)]
            for a in range(CT):
                nc.sync.dma_start(wv[a], Wa[a * 128:(a + 1) * 128, col0:col0 + 256])
            for st in range(NST):
                pv = ps_proj.tile([128, 512], dt.float32, tag="proj")
                for a in range(CT):
                    nc.tensor.matmul(pv[:, 0:256], xt[:, a, st * 128:(st + 1) * 128],
                                     wv[a], start=(a == 0), stop=(a == CT - 1))
                nc.vector.tensor_copy(v[:, st, :, 0:64], pv[:, 0:256])
            nc.vector.tensor_copy(v[:, :, :, 64:65], ones64)

            # ---- attention for the round's 4 heads ----
            for lh_i in range(HPR):
                lh = r * HPR + lh_i          # local head 0..7
                kt_h = lh_i // 2
                o = (lh_i % 2) * 64          # partition offset in kT/qT
                ct_g = lh // 2               # attnT ch-tile
                o2 = (lh % 2) * 64           # partition offset in attnT
                even = (lh % 2) == 0
                for qt in range(NQT):
                    t_q = qt * 512
                    n_s = (t_q + 512) // 128
                    pvv = ps_pv.tile([128, 512], dt.float32, tag="pv")
                    for s in range(n_s):
                        kv0 = s * 128
                        qlo = 0 if kv0 < t_q else kv0 - t_q
                        psc = ps_sc.tile([128, 512], dt.float32, tag="sc")
                        nc.tensor.matmul(
                            psc[:, qlo:512],
                            kT[o:o + 64, kt_h, kv0:kv0 + 128],
                            qT[o:o + 64, kt_h, t_q + qlo:t_q + 512],
                            start=True, stop=True)
                        wt = wei.tile([128, 512], dt.float32r, tag="wt")
                        nc.scalar.activation(wt[:, qlo:512], psc[:, qlo:512],
                                             func=mybir.ActivationFunctionType.Exp,
                                             scale=float(D) ** -0.5)
                        if kv0 >= t_q:
                            nc.vector.tensor_tensor(
                                wt[:, qlo:qlo + 128], wt[:, qlo:qlo + 128], mask,
                                op=mybir.AluOpType.mult)
                        nc.tensor.matmul(
                            pvv[0:65, qlo:512],
                            v[:, s, lh_i, :],
                            wt[:, qlo:512],
                            start=(s == 0), stop=(s == n_s - 1))
                    # normalize: rows [0, 64) are outT, row 64 is the denom
                    dn = small.tile([1, 512], dt.float32, tag="dn")
                    nc.vector.tensor_copy(dn, pvv[64:65, :])
                    rc = small.tile([1, 512], dt.float32r, tag="rc")
                    with nc.allow_low_precision(reason="fp32r softmax denom"):
                        nc.vector.reciprocal(rc, dn)
                    prep = ps_misc.tile([64, 512], dt.float32, tag="misc")
                    nc.tensor.matmul(prep, ones_r[:, 0:64], rc, start=True, stop=True)
                    rep = small.tile([64, 512], dt.float32, tag="rep")
                    nc.vector.tensor_copy(rep, prep)
                    nc.vector.tensor_tensor(
                        attnT[o2:o2 + 64, ct_g, t_q:t_q + 512],
                        pvv[0:64, :], rep,
                        op=mybir.AluOpType.mult)

        # ---- output projection: partial out[t, :] = attnT.T @ Wp ----
        for oc in range(2):
            wpt = wpp.tile([128, HPC // 2, 512], dt.float32r)
            for a in range(HPC // 2):
                nc.sync.dma_start(wpt[:, a, :],
                                  Wp[a * 128:(a + 1) * 128, oc * 512:(oc + 1) * 512])
            for tt8 in range(T // 128):
                po = ps_misc.tile([128, 512], dt.float32, tag="misc")
                for a in range(HPC // 2):
                    nc.tensor.matmul(po, attnT[:, a, tt8 * 128:(tt8 + 1) * 128],
                                     wpt[:, a, :], start=(a == 0),
                                     stop=(a == HPC // 2 - 1))
                osb = outsb.tile([128, 512], dt.float32, tag="osb")
                nc.vector.tensor_copy(osb, po)
                nc.sync.dma_start(
                    Out[tt8 * 128:(tt8 + 1) * 128, oc * 512:(oc + 1) * 512], osb)

    nc.compile()
    return nc


def kernel(x, W_attn, W_proj, _trace=False):
    global _compiled, last_exec_time_ns
    x = np.asarray(x, dtype=np.float32)
    W_attn = np.asarray(W_attn, dtype=np.float32)
    W_proj = np.asarray(W_proj, dtype=np.float32)

    if _compiled is None:
        _compiled = _build()
    nc = _compiled

    mask_np = np.triu(np.ones((128, 128), dtype=np.float32))
    in_maps = []
    for c in range(N_CORES):
        b, hf = c // 2, c % 2
        xT_c = np.ascontiguousarray(x[b].T)
        Wa_c = np.ascontiguousarray(np.concatenate([
            W_attn[:, hf * 512:(hf + 1) * 512],              # Q cols
            W_attn[:, C + hf * 512:C + (hf + 1) * 512],      # K cols
            W_attn[:, 2 * C + hf * 512:2 * C + (hf + 1) * 512],  # V cols
        ], axis=1))
        Wp_c = np.ascontiguousarray(W_proj[hf * 512:(hf + 1) * 512, :])
        in_maps.append({"xT": xT_c, "Wa": Wa_c, "Wp": Wp_c, "Msk": mask_np})

    res = run_bass_kernel_spmd(nc, in_maps, list(range(N_CORES)), trace=_trace)
    last_exec_time_ns = res.exec_time_ns

    out = np.empty((B, T, C), dtype=np.float32)
    for b in range(B):
        out[b] = res.results[2 * b]["outp"] + res.results[2 * b + 1]["outp"]
    return out


# revision 6
# speedup vs baseline: 1.0270x; 1.0270x over previous
"""Causal self-attention (B=4, T=2048, C=1024, H=16) on 8 Trainium2 NeuronCores.

Sharding: data-parallel on batch x tensor-parallel on heads.
  core c -> batch b = c//2, head half hf = c%2 (8 heads of 16).
Each core: QKV projection for its heads (column-parallel W_attn),
flash-style causal attention fully on-chip, row-parallel W_proj
producing a partial [T, C] output; pairs of partials are summed at
unshard time. All 8 cores run an identical program (SPMD) on
different data.

Projections run in fp32r (TRN2 PE mode: fp32 storage, 11-bit-mantissa
products, bf16-rate throughput at moving-dim >= 256); attention
matmuls run in ATT_DT (bf16 by default - halves weight-load cost).

Layout: host feeds x^T per batch ([C, T]) so QKV projections need no
on-device transposes; attention runs in "scores-transposed" [kv, q]
layout; the softmax denominator comes free from a ones-column
appended to V; normalization uses a K=1 PE replication matmul.
"""
import numpy as np
from contextlib import ExitStack

import concourse.bacc as bacc
import concourse.tile as tile
from concourse import mybir
from concourse.bass_utils import run_bass_kernel_spmd

dt = mybir.dt

B, T, C, H = 4, 2048, 1024, 16
D = C // H          # 64
N_CORES = 8
HPC = H // 2        # heads per core = 8
ROUNDS = 4          # 2 heads per round
CT = C // 128       # 8 c-tiles (contraction for projections)
NQT = T // 512      # 4 q-tiles per head
NTT = T // 512      # 4 t-tiles for K/Q projection
NST = T // 128      # 16 s-tiles for V projection

ATT_DT = dt.bfloat16   # dtype for attention matmuls (kT/qT/v/wei)

_compiled = None
last_exec_time_ns = None


def _build():
    nc = bacc.Bacc("TRN2", target_bir_lowering=False, debug=False,
                   num_devices=N_CORES)

    xT = nc.dram_tensor("xT", [C, T], dt.float32r, kind="ExternalInput").ap()
    Wa = nc.dram_tensor("Wa", [C, 3 * 512], dt.float32r, kind="ExternalInput").ap()
    Wp = nc.dram_tensor("Wp", [512, C], dt.float32r, kind="ExternalInput").ap()
    Msk = nc.dram_tensor("Msk", [128, 128], dt.float32, kind="ExternalInput").ap()
    Out = nc.dram_tensor("outp", [T, C], dt.float32, kind="ExternalOutput").ap()

    with tile.TileContext(nc) as tc, ExitStack() as ctx:
        consts = ctx.enter_context(tc.tile_pool(name="consts", bufs=1))
        kpool = ctx.enter_context(tc.tile_pool(name="kpool", bufs=2))
        qpool = ctx.enter_context(tc.tile_pool(name="qpool", bufs=2))
        vpool = ctx.enter_context(tc.tile_pool(name="vpool", bufs=2))
        apool = ctx.enter_context(tc.tile_pool(name="apool", bufs=1))
        wkq = ctx.enter_context(tc.tile_pool(name="wkq", bufs=8))
        wvp = ctx.enter_context(tc.tile_pool(name="wvp", bufs=8))
        wpp = ctx.enter_context(tc.tile_pool(name="wpp", bufs=1))
        wei = ctx.enter_context(tc.tile_pool(name="wei", bufs=4))
        small = ctx.enter_context(tc.tile_pool(name="small", bufs=3))
        outsb = ctx.enter_context(tc.tile_pool(name="outsb", bufs=3))
        ps_proj = ctx.enter_context(tc.tile_pool(name="ps_proj", bufs=2, space="PSUM"))
        ps_sc = ctx.enter_context(tc.tile_pool(name="ps_sc", bufs=2, space="PSUM"))
        ps_pv = ctx.enter_context(tc.tile_pool(name="ps_pv", bufs=2, space="PSUM"))
        ps_misc = ctx.enter_context(tc.tile_pool(name="ps_misc", bufs=2, space="PSUM"))

        # ---- constants & x^T ----
        xt = consts.tile([128, CT, T], dt.float32r)
        for a in range(CT):
            for tq in range(4):
                nc.sync.dma_start(xt[:, a, tq * 512:(tq + 1) * 512],
                                  xT[a * 128:(a + 1) * 128, tq * 512:(tq + 1) * 512])
        mask_f = consts.tile([128, 128], dt.float32)
        nc.sync.dma_start(mask_f, Msk)
        mask = consts.tile([128, 128], ATT_DT)
        nc.vector.tensor_copy(mask, mask_f)
        ones_f = consts.tile([1, 128], dt.float32)
        nc.vector.memset(ones_f, 1.0)
        ones_r = consts.tile([1, 128], dt.float32r)
        nc.vector.tensor_copy(ones_r, ones_f)
        ones64 = consts.tile([128, NST * 4], dt.float32)
        nc.vector.memset(ones64, 1.0)

        # attnT: [C_local=512 ch -> 4 part-tiles, T] (head pair r at tile r)
        attnT = apool.tile([128, HPC // 2, T], dt.float32r)

        for r in range(ROUNDS):
            # ---- K^T and Q^T projections for the round's 2 heads (128 ch) ----
            kT = kpool.tile([128, T], ATT_DT)
            qT = qpool.tile([128, T], ATT_DT)
            colK = 512 + r * 128
            wk = [wkq.tile([128, 128], dt.float32r, tag="wk", name=f"wk_{r}_{a}")
                  for a in range(CT)]
            for a in range(CT):
                nc.sync.dma_start(wk[a], Wa[a * 128:(a + 1) * 128, colK:colK + 128])
            for tt in range(NTT):
                pk = ps_proj.tile([128, 512], dt.float32, tag="proj")
                for a in range(CT):
                    nc.tensor.matmul(pk, wk[a], xt[:, a, tt * 512:(tt + 1) * 512],
                                     start=(a == 0), stop=(a == CT - 1))
                nc.vector.tensor_copy(kT[:, tt * 512:(tt + 1) * 512], pk)
            colQ = r * 128
            wq = [wkq.tile([128, 128], dt.float32r, tag="wk", name=f"wq_{r}_{a}")
                  for a in range(CT)]
            for a in range(CT):
                nc.sync.dma_start(wq[a], Wa[a * 128:(a + 1) * 128, colQ:colQ + 128])
            for tt in range(NTT):
                pq = ps_proj.tile([128, 512], dt.float32, tag="proj")
                for a in range(CT):
                    nc.tensor.matmul(pq, wq[a], xt[:, a, tt * 512:(tt + 1) * 512],
                                     start=(a == 0), stop=(a == CT - 1))
                nc.vector.tensor_copy(qT[:, tt * 512:(tt + 1) * 512], pq)

            # ---- V projection for a round PAIR (4 heads, 256 cols, even r) ----
            if r % 2 == 0:
                v = vpool.tile([128, NST, 4, 65], ATT_DT)
                colV = 1024 + r * 128
                wv = [wvp.tile([128, 256], dt.float32r, tag="wv", name=f"wv_{r}_{a}")
                      for a in range(CT)]
                for a in range(CT):
                    nc.sync.dma_start(wv[a], Wa[a * 128:(a + 1) * 128, colV:colV + 256])
                for st in range(NST):
                    pv = ps_proj.tile([128, 512], dt.float32, tag="proj")
                    for a in range(CT):
                        nc.tensor.matmul(pv[:, 0:256], xt[:, a, st * 128:(st + 1) * 128],
                                         wv[a], start=(a == 0), stop=(a == CT - 1))
                    nc.vector.tensor_copy(v[:, st, :, 0:64], pv[:, 0:256])
                nc.vector.tensor_copy(v[:, :, :, 64:65], ones64)

            # ---- attention for the round's 2 heads ----
            for lh_i in range(2):
                lh = 2 * r + lh_i            # local head 0..7
                o = lh_i * 64                # partition offset in kT/qT
                pv_i = lh % 4                # index within the round-pair v tile
                for qt in range(NQT):
                    t_q = qt * 512
                    n_s = (t_q + 512) // 128
                    pvv = ps_pv.tile([128, 512], dt.float32, tag="pv")
                    for s in range(n_s):
                        kv0 = s * 128
                        qlo = 0 if kv0 < t_q else kv0 - t_q
                        psc = ps_sc.tile([128, 512], dt.float32, tag="sc")
                        nc.tensor.matmul(
                            psc[:, qlo:512],
                            kT[o:o + 64, kv0:kv0 + 128],
                            qT[o:o + 64, t_q + qlo:t_q + 512],
                            start=True, stop=True)
                        wt = wei.tile([128, 512], ATT_DT, tag="wt")
                        nc.scalar.activation(wt[:, qlo:512], psc[:, qlo:512],
                                             func=mybir.ActivationFunctionType.Exp,
                                             scale=float(D) ** -0.5)
                        if kv0 >= t_q:
                            nc.vector.tensor_tensor(
                                wt[:, qlo:qlo + 128], wt[:, qlo:qlo + 128], mask,
                                op=mybir.AluOpType.mult)
                        nc.tensor.matmul(
                            pvv[0:65, qlo:512],
                            v[:, s, pv_i, :],
                            wt[:, qlo:512],
                            start=(s == 0), stop=(s == n_s - 1))
                    # normalize: rows [0, 64) are outT, row 64 is the denom
                    dn = small.tile([1, 512], dt.float32, tag="dn")
                    nc.vector.tensor_copy(dn, pvv[64:65, :])
                    rc = small.tile([1, 512], dt.float32r, tag="rc")
                    with nc.allow_low_precision(reason="fp32r softmax denom"):
                        nc.vector.reciprocal(rc, dn)
                    prep = ps_misc.tile([64, 512], dt.float32, tag="misc")
                    nc.tensor.matmul(prep, ones_r[:, 0:64], rc, start=True, stop=True)
                    rep = small.tile([64, 512], dt.float32, tag="rep")
                    nc.vector.tensor_copy(rep, prep)
                    nc.vector.tensor_tensor(
                        attnT[o:o + 64, r, t_q:t_q + 512],
                        pvv[0:64, :], rep,
                        op=mybir.AluOpType.mult)

        # ---- output projection: partial out[t, :] = attnT.T @ Wp ----
        for oc in range(2):
            wpt = wpp.tile([128, HPC // 2, 512], dt.float32r)
            for a in range(HPC // 2):
                nc.sync.dma_start(wpt[:, a, :],
                                  Wp[a * 128:(a + 1) * 128, oc * 512:(oc + 1) * 512])
            for tt8 in range(T // 128):
                po = ps_misc.tile([128, 512], dt.float32, tag="misc")
                for a in range(HPC // 2):
                    nc.tensor.matmul(po, attnT[:, a, tt8 * 128:(tt8 + 1) * 128],
                                     wpt[:, a, :], start=(a == 0),
                                     stop=(a == HPC // 2 - 1))
                osb = outsb.tile([128, 512], dt.float32, tag="osb")
                nc.vector.tensor_copy(osb, po)
                nc.sync.dma_start(
                    Out[tt8 * 128:(tt8 + 1) * 128, oc * 512:(oc + 1) * 512], osb)

    nc.compile()
    return nc


def kernel(x, W_attn, W_proj, _trace=False):
    global _compiled, last_exec_time_ns
    x = np.asarray(x, dtype=np.float32)
    W_attn = np.asarray(W_attn, dtype=np.float32)
    W_proj = np.asarray(W_proj, dtype=np.float32)

    if _compiled is None:
        _compiled = _build()
    nc = _compiled

    mask_np = np.triu(np.ones((128, 128), dtype=np.float32))
    in_maps = []
    for c in range(N_CORES):
        b, hf = c // 2, c % 2
        xT_c = np.ascontiguousarray(x[b].T)
        Wa_c = np.ascontiguousarray(np.concatenate([
            W_attn[:, hf * 512:(hf + 1) * 512],                  # Q cols
            W_attn[:, C + hf * 512:C + (hf + 1) * 512],          # K cols
            W_attn[:, 2 * C + hf * 512:2 * C + (hf + 1) * 512],  # V cols
        ], axis=1))
        Wp_c = np.ascontiguousarray(W_proj[hf * 512:(hf + 1) * 512, :])
        in_maps.append({"xT": xT_c, "Wa": Wa_c, "Wp": Wp_c, "Msk": mask_np})

    res = run_bass_kernel_spmd(nc, in_maps, list(range(N_CORES)), trace=_trace)
    last_exec_time_ns = res.exec_time_ns

    out = np.empty((B, T, C), dtype=np.float32)
    for b in range(B):
        out[b] = res.results[2 * b]["outp"] + res.results[2 * b + 1]["outp"]
    return out


# revision 12
# speedup vs baseline: 1.2988x; 1.2647x over previous
"""Causal self-attention (B=4, T=2048, C=1024, H=16) on 8 Trainium2 NeuronCores.

Sharding: data-parallel on batch x tensor-parallel on heads.
  core c -> batch b = c//2, head half hf = c%2 (8 heads of 16).
Each core: QKV projection for its heads (column-parallel W_attn),
flash-style causal attention fully on-chip, row-parallel W_proj
producing a partial [T, C] output; pairs of partials are summed at
unshard time. All 8 cores run an identical program (SPMD) on
different data.

Projections run in fp32r (TRN2 PE mode: fp32 storage, 11-bit-mantissa
products, bf16-rate throughput at moving-dim >= 256); attention
matmuls run in bf16.

Structure: emission is software-pipelined - projection matmul groups
of round r+1 are emitted between attention units of round r so the
PE instruction stream stays dense (keeps the HAM clock gate at full
rate); inside a unit, scores matmuls run one kv-tile ahead of the
weiV matmuls to hide the exp latency. Softmax normalization is
deferred: unnormalized outputs and denominators accumulate, then one
batched reciprocal + per-tile scale pass runs before the output
projection.

Layout: host feeds x^T per batch ([C, T]) so QKV projections need no
on-device transposes; attention runs in "scores-transposed" [kv, q]
layout; the softmax denominator comes free from a ones-column
appended to V; denominator replication uses small K=1/K=2 PE matmuls.
"""
import numpy as np
from contextlib import ExitStack

import concourse.bacc as bacc
import concourse.tile as tile
from concourse import mybir
from concourse.bass_utils import run_bass_kernel_spmd

dt = mybir.dt

B, T, C, H = 4, 2048, 1024, 16
D = C // H          # 64
N_CORES = 8
HPC = H // 2        # heads per core = 8
ROUNDS = 4          # 2 heads per round
CT = C // 128       # 8 c-tiles (contraction for projections)
NQT = T // 512      # 4 q-tiles per head
NTT = T // 512      # 4 t-tiles for K/Q projection
NST = T // 128      # 16 s-tiles for V projection

ATT_DT = dt.bfloat16   # dtype for attention matmuls (kT/qT/v/wei)

_compiled = None
last_exec_time_ns = None


def _build():
    nc = bacc.Bacc("TRN2", target_bir_lowering=False, debug=False,
                   num_devices=N_CORES)

    xT = nc.dram_tensor("xT", [C, T], dt.float32r, kind="ExternalInput").ap()
    Wa = nc.dram_tensor("Wa", [C, 3 * 512], dt.float32r, kind="ExternalInput").ap()
    Wp = nc.dram_tensor("Wp", [512, C], dt.float32r, kind="ExternalInput").ap()
    Msk = nc.dram_tensor("Msk", [128, 128], dt.float32, kind="ExternalInput").ap()
    Out = nc.dram_tensor("outp", [T, C], dt.float32, kind="ExternalOutput").ap()

    with tile.TileContext(nc) as tc, ExitStack() as ctx:
        consts = ctx.enter_context(tc.tile_pool(name="consts", bufs=1))
        kpool = ctx.enter_context(tc.tile_pool(name="kpool", bufs=2))
        qpool = ctx.enter_context(tc.tile_pool(name="qpool", bufs=2))
        vpool = ctx.enter_context(tc.tile_pool(name="vpool", bufs=2))
        apool = ctx.enter_context(tc.tile_pool(name="apool", bufs=1))
        wkq = ctx.enter_context(tc.tile_pool(name="wkq", bufs=16))
        wvp = ctx.enter_context(tc.tile_pool(name="wvp", bufs=8))
        wpp = ctx.enter_context(tc.tile_pool(name="wpp", bufs=1))
        wei = ctx.enter_context(tc.tile_pool(name="wei", bufs=6))
        small = ctx.enter_context(tc.tile_pool(name="small", bufs=3))
        outsb = ctx.enter_context(tc.tile_pool(name="outsb", bufs=3))
        ps_proj = ctx.enter_context(tc.tile_pool(name="ps_proj", bufs=2, space="PSUM"))
        ps_sc = ctx.enter_context(tc.tile_pool(name="ps_sc", bufs=2, space="PSUM"))
        ps_pv = ctx.enter_context(tc.tile_pool(name="ps_pv", bufs=2, space="PSUM"))
        ps_misc = ctx.enter_context(tc.tile_pool(name="ps_misc", bufs=2, space="PSUM"))

        # ---- constants & x^T ----
        xt = consts.tile([128, CT, T], dt.float32r)
        for a in range(CT):
            for tq in range(4):
                nc.sync.dma_start(xt[:, a, tq * 512:(tq + 1) * 512],
                                  xT[a * 128:(a + 1) * 128, tq * 512:(tq + 1) * 512])
        mask_f = consts.tile([128, 128], dt.float32)
        nc.sync.dma_start(mask_f, Msk)
        mask = consts.tile([128, 128], ATT_DT)
        nc.vector.tensor_copy(mask, mask_f)
        ones64 = consts.tile([128, NST * 4], dt.float32)
        nc.vector.memset(ones64, 1.0)
        ones_f = consts.tile([1, 128], dt.float32)
        nc.vector.memset(ones_f, 1.0)
        ones_r = consts.tile([1, 128], dt.float32r)
        nc.vector.tensor_copy(ones_r, ones_f)

        # unnormalized attn^T and per-(head, t) softmax denominators;
        # head lh's denominators live at partition 32*(lh//2), plane lh%2
        # (single-partition engine accesses must be 32-aligned)
        attnT = apool.tile([128, HPC // 2, T], dt.float32r)
        dns = consts.tile([128, 2, T], dt.float32)

        kTs, qTs, vs = {}, {}, {}

        def proj_chunks(r):
            """Emit W DMAs; return list of closures, each emitting one
            PSUM matmul group of round r's projections."""
            chunks = []
            kT = kpool.tile([128, T], ATT_DT, name=f"kT_{r}", tag="kT")
            qT = qpool.tile([128, T], ATT_DT, name=f"qT_{r}", tag="qT")
            kTs[r], qTs[r] = kT, qT
            colK = 512 + r * 128
            wk = [wkq.tile([128, 128], dt.float32r, tag="wk", name=f"wk_{r}_{a}")
                  for a in range(CT)]
            colQ = r * 128
            wq = [wkq.tile([128, 128], dt.float32r, tag="wk", name=f"wq_{r}_{a}")
                  for a in range(CT)]
            for a in range(CT):
                nc.sync.dma_start(wk[a], Wa[a * 128:(a + 1) * 128, colK:colK + 128])
                nc.sync.dma_start(wq[a], Wa[a * 128:(a + 1) * 128, colQ:colQ + 128])
            if r % 2 == 0:
                v = vpool.tile([128, NST, 4, 65], ATT_DT, name=f"v_{r}", tag="v")
                vs[r // 2] = v
                colV = 1024 + r * 128
                wv = [wvp.tile([128, 256], dt.float32r, tag="wv", name=f"wv_{r}_{a}")
                      for a in range(CT)]
                for a in range(CT):
                    nc.sync.dma_start(wv[a],
                                      Wa[a * 128:(a + 1) * 128, colV:colV + 256])

            def kq_group(w, dst, tt):
                def emit():
                    p = ps_proj.tile([128, 512], dt.float32, tag="proj",
                                     name=f"pp_{r}_{tt}")
                    for a in range(CT):
                        nc.tensor.matmul(p, w[a], xt[:, a, tt * 512:(tt + 1) * 512],
                                         start=(a == 0), stop=(a == CT - 1))
                    nc.vector.tensor_copy(dst[:, tt * 512:(tt + 1) * 512], p)
                return emit

            def v_group(st):
                def emit():
                    p = ps_proj.tile([128, 512], dt.float32, tag="proj",
                                     name=f"pv_{r}_{st}")
                    for a in range(CT):
                        nc.tensor.matmul(p[:, 0:256],
                                         xt[:, a, st * 128:(st + 1) * 128],
                                         wv[a], start=(a == 0), stop=(a == CT - 1))
                    nc.vector.tensor_copy(v[:, st, :, 0:64], p[:, 0:256])
                    if st == NST - 1:
                        nc.vector.tensor_copy(v[:, :, :, 64:65], ones64)
                return emit

            for tt in range(NTT):
                chunks.append(kq_group(wk, kT, tt))
            for tt in range(NTT):
                chunks.append(kq_group(wq, qT, tt))
            if r % 2 == 0:
                for st in range(NST):
                    chunks.append(v_group(st))
            return chunks

        def attention_unit(r, lh_i, qt):
            kT, qT, v = kTs[r], qTs[r], vs[r // 2]
            lh = 2 * r + lh_i
            o = lh_i * 64
            pv_i = lh % 4
            t_q = qt * 512
            n_s = (t_q + 512) // 128
            pvv = ps_pv.tile([128, 512], dt.float32, tag="pv",
                             name=f"pvv_{lh}_{qt}")
            wts = {}

            def do_scores(s):
                kv0 = s * 128
                qlo = 0 if kv0 < t_q else kv0 - t_q
                psc = ps_sc.tile([128, 512], dt.float32, tag="sc",
                                 name=f"psc_{lh}_{qt}_{s}")
                nc.tensor.matmul(
                    psc[:, qlo:512],
                    kT[o:o + 64, kv0:kv0 + 128],
                    qT[o:o + 64, t_q + qlo:t_q + 512],
                    start=True, stop=True)
                wt = wei.tile([128, 512], ATT_DT, tag="wt",
                              name=f"wt_{lh}_{qt}_{s}")
                nc.scalar.activation(wt[:, qlo:512], psc[:, qlo:512],
                                     func=mybir.ActivationFunctionType.Exp,
                                     scale=float(D) ** -0.5)
                if kv0 >= t_q:
                    nc.vector.tensor_tensor(
                        wt[:, qlo:qlo + 128], wt[:, qlo:qlo + 128], mask,
                        op=mybir.AluOpType.mult)
                wts[s] = (wt, qlo)

            def do_wv(s):
                wt, qlo = wts.pop(s)
                nc.tensor.matmul(
                    pvv[0:65, qlo:512],
                    v[:, s, pv_i, :],
                    wt[:, qlo:512],
                    start=(s == 0), stop=(s == n_s - 1))

            for s in range(n_s):
                do_scores(s)
                if s >= 1:
                    do_wv(s - 1)
            do_wv(n_s - 1)

            # stash unnormalized out^T and the denominator row
            nc.vector.tensor_copy(attnT[o:o + 64, r, t_q:t_q + 512], pvv[0:64, :])
            g2 = 32 * (lh // 2)
            nc.vector.tensor_copy(dns[g2:g2 + 1, lh % 2, t_q:t_q + 512],
                                  pvv[64:65, :])

        # ---- software-pipelined emission ----
        for c in proj_chunks(0):
            c()
        for r in range(ROUNDS):
            nxt = proj_chunks(r + 1) if r + 1 < ROUNDS else []
            units = [(lh_i, qt) for lh_i in range(2) for qt in range(NQT)]
            per = (len(nxt) + len(units) - 1) // len(units) if nxt else 0
            for i, (lh_i, qt) in enumerate(units):
                attention_unit(r, lh_i, qt)
                for c in nxt[i * per:(i + 1) * per]:
                    c()

        # ---- batched softmax normalization ----
        # reciprocal via ACT: 1/d = exp(-ln(d)); in-place on dns
        nc.scalar.activation(dns, dns, func=mybir.ActivationFunctionType.Ln)
        nc.scalar.activation(dns, dns, func=mybir.ActivationFunctionType.Exp,
                             scale=-1.0)
        for g in range(HPC // 2):
            for tt in range(NTT):
                rep = small.tile([128, 512], dt.float32, tag="rep",
                                 name=f"rep_{g}_{tt}")
                for par in range(2):
                    rc1 = small.tile([1, 512], dt.float32r, tag=f"rc1_{par}",
                                     name=f"rc1_{g}_{tt}_{par}")
                    nc.vector.tensor_copy(
                        rc1, dns[32 * g:32 * g + 1, par, tt * 512:(tt + 1) * 512])
                    prep = ps_misc.tile([64, 512], dt.float32, tag="misc",
                                        name=f"prep_{g}_{tt}_{par}")
                    nc.tensor.matmul(prep, ones_r[:, 0:64], rc1,
                                     start=True, stop=True)
                    nc.vector.tensor_copy(rep[64 * par:64 * par + 64, :], prep)
                nc.vector.tensor_tensor(
                    attnT[:, g, tt * 512:(tt + 1) * 512],
                    attnT[:, g, tt * 512:(tt + 1) * 512], rep,
                    op=mybir.AluOpType.mult)

        # ---- output projection: partial out[t, :] = attnT.T @ Wp ----
        for oc in range(2):
            wpt = wpp.tile([128, HPC // 2, 512], dt.float32r, name=f"wpt_{oc}",
                           tag="wpt")
            for a in range(HPC // 2):
                nc.sync.dma_start(wpt[:, a, :],
                                  Wp[a * 128:(a + 1) * 128, oc * 512:(oc + 1) * 512])
            for tt8 in range(T // 128):
                po = ps_misc.tile([128, 512], dt.float32, tag="misc",
                                  name=f"po_{oc}_{tt8}")
                for a in range(HPC // 2):
                    nc.tensor.matmul(po, attnT[:, a, tt8 * 128:(tt8 + 1) * 128],
                                     wpt[:, a, :], start=(a == 0),
                                     stop=(a == HPC // 2 - 1))
                osb = outsb.tile([128, 512], dt.float32, tag="osb",
                                 name=f"osb_{oc}_{tt8}")
                nc.vector.tensor_copy(osb, po)
                nc.sync.dma_start(
                    Out[tt8 * 128:(tt8 + 1) * 128, oc * 512:(oc + 1) * 512], osb)

    nc.compile()
    return nc


def kernel(x, W_attn, W_proj, _trace=False):
    global _compiled, last_exec_time_ns
    x = np.asarray(x, dtype=np.float32)
    W_attn = np.asarray(W_attn, dtype=np.float32)
    W_proj = np.asarray(W_proj, dtype=np.float32)

    if _compiled is None:
        _compiled = _build()
    nc = _compiled

    mask_np = np.triu(np.ones((128, 128), dtype=np.float32))
    in_maps = []
    for c in range(N_CORES):
        b, hf = c // 2, c % 2
        xT_c = np.ascontiguousarray(x[b].T)
        Wa_c = np.ascontiguousarray(np.concatenate([
            W_attn[:, hf * 512:(hf + 1) * 512],                  # Q cols
            W_attn[:, C + hf * 512:C + (hf + 1) * 512],          # K cols
            W_attn[:, 2 * C + hf * 512:2 * C + (hf + 1) * 512],  # V cols
        ], axis=1))
        Wp_c = np.ascontiguousarray(W_proj[hf * 512:(hf + 1) * 512, :])
        in_maps.append({"xT": xT_c, "Wa": Wa_c, "Wp": Wp_c, "Msk": mask_np})

    res = run_bass_kernel_spmd(nc, in_maps, list(range(N_CORES)), trace=_trace)
    last_exec_time_ns = res.exec_time_ns

    out = np.empty((B, T, C), dtype=np.float32)
    for b in range(B):
        out[b] = res.results[2 * b]["outp"] + res.results[2 * b + 1]["outp"]
    return out


# revision 15
# speedup vs baseline: 1.4064x; 1.0828x over previous
"""Causal self-attention (B=4, T=2048, C=1024, H=16) on 8 Trainium2 NeuronCores.

Sharding: data-parallel on batch x tensor-parallel on heads.
  core c -> batch b = c//2, head half hf = c%2 (8 heads of 16).
Each core: QKV projection for its heads (column-parallel W_attn),
flash-style causal attention fully on-chip, row-parallel W_proj
producing a partial [T, C] output; pairs of partials are summed at
unshard time. All 8 cores run an identical program (SPMD) on
different data.

Projections run in fp32r (TRN2 PE mode: fp32 storage, 11-bit-mantissa
products, bf16-rate throughput at moving-dim >= 256); attention
matmuls run in bf16.

Structure: emission is software-pipelined - projection matmul groups
of round r+1 are emitted between attention units of round r so the
PE instruction stream stays dense (keeps the HAM clock gate at full
rate); inside a unit, scores matmuls run one kv-tile ahead of the
weiV matmuls to hide the exp latency. Softmax normalization is
deferred: unnormalized outputs and denominators accumulate, then one
batched reciprocal + per-tile scale pass runs before the output
projection.

Layout: host feeds x^T per batch ([C, T]) so QKV projections need no
on-device transposes; attention runs in "scores-transposed" [kv, q]
layout; the softmax denominator comes free from a ones-column
appended to V; denominator replication uses small K=1/K=2 PE matmuls.
"""
import numpy as np
from contextlib import ExitStack

import concourse.bacc as bacc
import concourse.tile as tile
from concourse import mybir
from concourse.bass_utils import run_bass_kernel_spmd

dt = mybir.dt

B, T, C, H = 4, 2048, 1024, 16
D = C // H          # 64
N_CORES = 8
HPC = H // 2        # heads per core = 8
ROUNDS = 4          # 2 heads per round
CT = C // 128       # 8 c-tiles (contraction for projections)
NQT = T // 512      # 4 q-tiles per head
NTT = T // 512      # 4 t-tiles for K/Q projection
NST = T // 128      # 16 s-tiles for V projection

ATT_DT = dt.bfloat16   # dtype for attention matmuls (kT/qT/v/wei)

_compiled = None
last_exec_time_ns = None


def _build():
    nc = bacc.Bacc("TRN2", target_bir_lowering=False, debug=False,
                   num_devices=N_CORES)

    xT = nc.dram_tensor("xT", [C, T], dt.float32r, kind="ExternalInput").ap()
    Wa = nc.dram_tensor("Wa", [C, 3 * 512], dt.float32r, kind="ExternalInput").ap()
    Wp = nc.dram_tensor("Wp", [512, C], dt.float32r, kind="ExternalInput").ap()
    Msk = nc.dram_tensor("Msk", [128, 128], dt.float32, kind="ExternalInput").ap()
    Out = nc.dram_tensor("outp", [T, C], dt.float32, kind="ExternalOutput").ap()

    with tile.TileContext(nc) as tc, ExitStack() as ctx:
        consts = ctx.enter_context(tc.tile_pool(name="consts", bufs=1))
        kpool = ctx.enter_context(tc.tile_pool(name="kpool", bufs=2))
        qpool = ctx.enter_context(tc.tile_pool(name="qpool", bufs=2))
        vpool = ctx.enter_context(tc.tile_pool(name="vpool", bufs=2))
        apool = ctx.enter_context(tc.tile_pool(name="apool", bufs=1))
        wkq = ctx.enter_context(tc.tile_pool(name="wkq", bufs=16))
        wvp = ctx.enter_context(tc.tile_pool(name="wvp", bufs=8))
        wpp = ctx.enter_context(tc.tile_pool(name="wpp", bufs=2))
        wei = ctx.enter_context(tc.tile_pool(name="wei", bufs=6))
        small = ctx.enter_context(tc.tile_pool(name="small", bufs=3))
        outsb = ctx.enter_context(tc.tile_pool(name="outsb", bufs=2))
        ps_proj = ctx.enter_context(tc.tile_pool(name="ps_proj", bufs=1, space="PSUM"))
        ps_sc = ctx.enter_context(tc.tile_pool(name="ps_sc", bufs=3, space="PSUM"))
        ps_pv = ctx.enter_context(tc.tile_pool(name="ps_pv", bufs=2, space="PSUM"))
        ps_misc = ctx.enter_context(tc.tile_pool(name="ps_misc", bufs=2, space="PSUM"))

        # ---- constants & x^T ----
        xt = consts.tile([128, CT, T], dt.float32r)
        for a in range(CT):
            for tq in range(4):
                nc.sync.dma_start(xt[:, a, tq * 512:(tq + 1) * 512],
                                  xT[a * 128:(a + 1) * 128, tq * 512:(tq + 1) * 512])
        mask_f = consts.tile([128, 128], dt.float32)
        nc.sync.dma_start(mask_f, Msk)
        mask = consts.tile([128, 128], ATT_DT)
        nc.vector.tensor_copy(mask, mask_f)
        ones64 = consts.tile([128, NST * 4], dt.float32)
        nc.vector.memset(ones64, 1.0)
        ones_f = consts.tile([1, 128], dt.float32)
        nc.vector.memset(ones_f, 1.0)
        ones_r = consts.tile([1, 128], dt.float32r)
        nc.vector.tensor_copy(ones_r, ones_f)

        # unnormalized attn^T and per-(head, t) softmax denominators;
        # head lh's denominators live at partition 32*(lh//2), plane lh%2
        # (single-partition engine accesses must be 32-aligned)
        attnT = apool.tile([128, HPC // 2, T], dt.float32r)
        dns = consts.tile([128, 2, T], dt.float32)

        kTs, qTs, vs = {}, {}, {}

        def proj_chunks(r):
            """Emit W DMAs; return list of closures, each emitting one
            PSUM matmul group of round r's projections."""
            chunks = []
            kT = kpool.tile([128, T], ATT_DT, name=f"kT_{r}", tag="kT")
            qT = qpool.tile([128, T], ATT_DT, name=f"qT_{r}", tag="qT")
            kTs[r], qTs[r] = kT, qT
            colK = 512 + r * 128
            wk = [wkq.tile([128, 128], dt.float32r, tag="wk", name=f"wk_{r}_{a}")
                  for a in range(CT)]
            colQ = r * 128
            wq = [wkq.tile([128, 128], dt.float32r, tag="wk", name=f"wq_{r}_{a}")
                  for a in range(CT)]
            for a in range(CT):
                nc.sync.dma_start(wk[a], Wa[a * 128:(a + 1) * 128, colK:colK + 128])
                nc.sync.dma_start(wq[a], Wa[a * 128:(a + 1) * 128, colQ:colQ + 128])
            if r % 2 == 0:
                v = vpool.tile([128, NST, 4, 65], ATT_DT, name=f"v_{r}", tag="v")
                vs[r // 2] = v
                colV = 1024 + r * 128
                wv = [wvp.tile([128, 256], dt.float32r, tag="wv", name=f"wv_{r}_{a}")
                      for a in range(CT)]
                for a in range(CT):
                    nc.sync.dma_start(wv[a],
                                      Wa[a * 128:(a + 1) * 128, colV:colV + 256])

            def kq_group(w, dst, tt):
                def emit():
                    p = ps_proj.tile([128, 512], dt.float32, tag="proj",
                                     name=f"pp_{r}_{tt}")
                    for a in range(CT):
                        nc.tensor.matmul(p, w[a], xt[:, a, tt * 512:(tt + 1) * 512],
                                         start=(a == 0), stop=(a == CT - 1))
                    nc.vector.tensor_copy(dst[:, tt * 512:(tt + 1) * 512], p)
                return emit

            def v_group(st):
                def emit():
                    p = ps_proj.tile([128, 512], dt.float32, tag="proj",
                                     name=f"pv_{r}_{st}")
                    for a in range(CT):
                        nc.tensor.matmul(p[:, 0:256],
                                         xt[:, a, st * 128:(st + 1) * 128],
                                         wv[a], start=(a == 0), stop=(a == CT - 1))
                    nc.vector.tensor_copy(v[:, st, :, 0:64], p[:, 0:256])
                    if st == NST - 1:
                        nc.vector.tensor_copy(v[:, :, :, 64:65], ones64)
                return emit

            for tt in range(NTT):
                chunks.append(kq_group(wk, kT, tt))
            for tt in range(NTT):
                chunks.append(kq_group(wq, qT, tt))
            if r % 2 == 0:
                for st in range(NST):
                    chunks.append(v_group(st))
            return chunks

        def attention_unit(r, lh_i, qt):
            kT, qT, v = kTs[r], qTs[r], vs[r // 2]
            lh = 2 * r + lh_i
            o = lh_i * 64
            pv_i = lh % 4
            t_q = qt * 512
            n_s = (t_q + 512) // 128
            pvv = ps_pv.tile([128, 512], dt.float32, tag="pv",
                             name=f"pvv_{lh}_{qt}")
            wts = {}

            def do_scores(s):
                kv0 = s * 128
                qlo = 0 if kv0 < t_q else kv0 - t_q
                psc = ps_sc.tile([128, 512], dt.float32, tag="sc",
                                 name=f"psc_{lh}_{qt}_{s}")
                nc.tensor.matmul(
                    psc[:, qlo:512],
                    kT[o:o + 64, kv0:kv0 + 128],
                    qT[o:o + 64, t_q + qlo:t_q + 512],
                    start=True, stop=True)
                wt = wei.tile([128, 512], ATT_DT, tag="wt",
                              name=f"wt_{lh}_{qt}_{s}")
                nc.scalar.activation(wt[:, qlo:512], psc[:, qlo:512],
                                     func=mybir.ActivationFunctionType.Exp,
                                     scale=float(D) ** -0.5)
                if kv0 >= t_q:
                    nc.vector.tensor_tensor(
                        wt[:, qlo:qlo + 128], wt[:, qlo:qlo + 128], mask,
                        op=mybir.AluOpType.mult)
                wts[s] = (wt, qlo)

            def do_wv(s):
                wt, qlo = wts.pop(s)
                nc.tensor.matmul(
                    pvv[0:65, qlo:512],
                    v[:, s, pv_i, :],
                    wt[:, qlo:512],
                    start=(s == 0), stop=(s == n_s - 1))

            for s in range(n_s):
                do_scores(s)
                if s >= 1:
                    do_wv(s - 1)
            do_wv(n_s - 1)

            # stash unnormalized out^T and the denominator row
            nc.vector.tensor_copy(attnT[o:o + 64, r, t_q:t_q + 512], pvv[0:64, :])
            g2 = 32 * (lh // 2)
            nc.vector.tensor_copy(dns[g2:g2 + 1, lh % 2, t_q:t_q + 512],
                                  pvv[64:65, :])

        def normalize_chunk(g, tt):
            rep = small.tile([128, 512], dt.float32, tag="rep",
                             name=f"rep_{g}_{tt}")
            for par in range(2):
                rc1 = small.tile([1, 512], dt.float32r, tag=f"rc1_{par}",
                                 name=f"rc1_{g}_{tt}_{par}")
                nc.vector.tensor_copy(
                    rc1, dns[32 * g:32 * g + 1, par, tt * 512:(tt + 1) * 512])
                prep = ps_misc.tile([64, 512], dt.float32, tag="misc",
                                    name=f"prep_{g}_{tt}_{par}")
                nc.tensor.matmul(prep, ones_r[:, 0:64], rc1,
                                 start=True, stop=True)
                nc.vector.tensor_copy(rep[64 * par:64 * par + 64, :], prep)
            nc.vector.tensor_tensor(
                attnT[:, g, tt * 512:(tt + 1) * 512],
                attnT[:, g, tt * 512:(tt + 1) * 512], rep,
                op=mybir.AluOpType.mult)

        def outproj_chunk(oc, tt8):
            po = ps_misc.tile([128, 512], dt.float32, tag="misc",
                              name=f"po_{oc}_{tt8}")
            for a in range(HPC // 2):
                nc.tensor.matmul(po, attnT[:, a, tt8 * 128:(tt8 + 1) * 128],
                                 wpts[oc][:, a, :], start=(a == 0),
                                 stop=(a == HPC // 2 - 1))
            osb = outsb.tile([128, 512], dt.float32, tag="osb",
                             name=f"osb_{oc}_{tt8}")
            nc.vector.tensor_copy(osb, po)
            nc.sync.dma_start(
                Out[tt8 * 128:(tt8 + 1) * 128, oc * 512:(oc + 1) * 512], osb)

        def tail_chunks(tt):
            """Everything that becomes ready once all heads finished q-tile
            tt: denominator reciprocal slice, normalization, out-proj."""
            # reciprocal via ACT: 1/d = exp(-ln(d)); in-place on dns slice
            sl = dns[:, :, tt * 512:(tt + 1) * 512]
            nc.scalar.activation(sl, sl, func=mybir.ActivationFunctionType.Ln)
            nc.scalar.activation(sl, sl, func=mybir.ActivationFunctionType.Exp,
                                 scale=-1.0)
            for g in range(HPC // 2):
                normalize_chunk(g, tt)
            for oc in range(2):
                for tt8 in range(4 * tt, 4 * tt + 4):
                    outproj_chunk(oc, tt8)

        # ---- software-pipelined emission ----
        wpts = {}
        for c in proj_chunks(0):
            c()
        for r in range(ROUNDS):
            if r == ROUNDS - 1:
                for oc in range(2):
                    wpt = wpp.tile([128, HPC // 2, 512], dt.float32r,
                                   name=f"wpt_{oc}", tag="wpt")
                    wpts[oc] = wpt
                    for a in range(HPC // 2):
                        nc.sync.dma_start(
                            wpt[:, a, :],
                            Wp[a * 128:(a + 1) * 128, oc * 512:(oc + 1) * 512])
            nxt = proj_chunks(r + 1) if r + 1 < ROUNDS else []
            units = [(lh_i, qt) for qt in range(NQT) for lh_i in range(2)]
            per = (len(nxt) + len(units) - 1) // len(units) if nxt else 0
            for i, (lh_i, qt) in enumerate(units):
                attention_unit(r, lh_i, qt)
                for c in nxt[i * per:(i + 1) * per]:
                    c()
                if r == ROUNDS - 1 and lh_i == 1:
                    tail_chunks(qt)

    nc.compile()
    return nc


def kernel(x, W_attn, W_proj, _trace=False):
    global _compiled, last_exec_time_ns
    x = np.asarray(x, dtype=np.float32)
    W_attn = np.asarray(W_attn, dtype=np.float32)
    W_proj = np.asarray(W_proj, dtype=np.float32)

    if _compiled is None:
        _compiled = _build()
    nc = _compiled

    mask_np = np.triu(np.ones((128, 128), dtype=np.float32))
    in_maps = []
    for c in range(N_CORES):
        b, hf = c // 2, c % 2
        xT_c = np.ascontiguousarray(x[b].T)
        Wa_c = np.ascontiguousarray(np.concatenate([
            W_attn[:, hf * 512:(hf + 1) * 512],                  # Q cols
            W_attn[:, C + hf * 512:C + (hf + 1) * 512],          # K cols
            W_attn[:, 2 * C + hf * 512:2 * C + (hf + 1) * 512],  # V cols
        ], axis=1))
        Wp_c = np.ascontiguousarray(W_proj[hf * 512:(hf + 1) * 512, :])
        in_maps.append({"xT": xT_c, "Wa": Wa_c, "Wp": Wp_c, "Msk": mask_np})

    res = run_bass_kernel_spmd(nc, in_maps, list(range(N_CORES)), trace=_trace)
    last_exec_time_ns = res.exec_time_ns

    out = np.empty((B, T, C), dtype=np.float32)
    for b in range(B):
        out[b] = res.results[2 * b]["outp"] + res.results[2 * b + 1]["outp"]
    return out


# revision 16
# speedup vs baseline: 1.4246x; 1.0129x over previous
"""Causal self-attention (B=4, T=2048, C=1024, H=16) on 8 Trainium2 NeuronCores.

Sharding: data-parallel on batch x tensor-parallel on heads.
  core c -> batch b = c//2, head half hf = c%2 (8 heads of 16).
Each core: QKV projection for its heads (column-parallel W_attn),
flash-style causal attention fully on-chip, row-parallel W_proj
producing a partial [T, C] output; pairs of partials are summed at
unshard time. All 8 cores run an identical program (SPMD) on
different data.

Projections run in fp32r (TRN2 PE mode: fp32 storage, 11-bit-mantissa
products, bf16-rate throughput at moving-dim >= 256); attention
matmuls run in bf16.

Structure: emission is software-pipelined - projection matmul groups
of round r+1 are emitted between attention units of round r so the
PE instruction stream stays dense (keeps the HAM clock gate at full
rate); inside a unit, scores matmuls run one kv-tile ahead of the
weiV matmuls to hide the exp latency. Softmax normalization is
deferred: unnormalized outputs and denominators accumulate, then one
batched reciprocal + per-tile scale pass runs before the output
projection.

Layout: host feeds x^T per batch ([C, T]) so QKV projections need no
on-device transposes; attention runs in "scores-transposed" [kv, q]
layout; the softmax denominator comes free from a ones-column
appended to V; denominator replication uses small K=1/K=2 PE matmuls.
"""
import numpy as np
from contextlib import ExitStack

import concourse.bacc as bacc
import concourse.tile as tile
from concourse import mybir
from concourse.bass_utils import run_bass_kernel_spmd

dt = mybir.dt

B, T, C, H = 4, 2048, 1024, 16
D = C // H          # 64
N_CORES = 8
HPC = H // 2        # heads per core = 8
ROUNDS = 4          # 2 heads per round
CT = C // 128       # 8 c-tiles (contraction for projections)
NQT = T // 512      # 4 q-tiles per head
NTT = T // 512      # 4 t-tiles for K/Q projection
NST = T // 128      # 16 s-tiles for V projection

ATT_DT = dt.bfloat16   # dtype for attention matmuls (kT/qT/v/wei)

_compiled = None
last_exec_time_ns = None


def _build():
    nc = bacc.Bacc("TRN2", target_bir_lowering=False, debug=False,
                   num_devices=N_CORES)

    xT = nc.dram_tensor("xT", [C, T], dt.float32r, kind="ExternalInput").ap()
    Wa = nc.dram_tensor("Wa", [C, 3 * 512], dt.float32r, kind="ExternalInput").ap()
    Wp = nc.dram_tensor("Wp", [512, C], dt.float32r, kind="ExternalInput").ap()
    Msk = nc.dram_tensor("Msk", [128, 128], dt.float32, kind="ExternalInput").ap()
    Out = nc.dram_tensor("outp", [T, C], dt.float32, kind="ExternalOutput").ap()

    with tile.TileContext(nc) as tc, ExitStack() as ctx:
        consts = ctx.enter_context(tc.tile_pool(name="consts", bufs=1))
        kpool = ctx.enter_context(tc.tile_pool(name="kpool", bufs=2))
        qpool = ctx.enter_context(tc.tile_pool(name="qpool", bufs=2))
        vpool = ctx.enter_context(tc.tile_pool(name="vpool", bufs=2))
        apool = ctx.enter_context(tc.tile_pool(name="apool", bufs=1))
        wkq = ctx.enter_context(tc.tile_pool(name="wkq", bufs=16))
        wvp = ctx.enter_context(tc.tile_pool(name="wvp", bufs=8))
        wpp = ctx.enter_context(tc.tile_pool(name="wpp", bufs=2))
        wei = ctx.enter_context(tc.tile_pool(name="wei", bufs=6))
        small = ctx.enter_context(tc.tile_pool(name="small", bufs=3))
        outsb = ctx.enter_context(tc.tile_pool(name="outsb", bufs=2))
        ps_proj = ctx.enter_context(tc.tile_pool(name="ps_proj", bufs=1, space="PSUM"))
        ps_sc = ctx.enter_context(tc.tile_pool(name="ps_sc", bufs=3, space="PSUM"))
        ps_pv = ctx.enter_context(tc.tile_pool(name="ps_pv", bufs=2, space="PSUM"))
        ps_misc = ctx.enter_context(tc.tile_pool(name="ps_misc", bufs=2, space="PSUM"))

        # ---- constants & x^T ----
        xt = consts.tile([128, CT, T], dt.float32r)
        for tq in range(4):
            for a in range(CT):
                nc.sync.dma_start(xt[:, a, tq * 512:(tq + 1) * 512],
                                  xT[a * 128:(a + 1) * 128, tq * 512:(tq + 1) * 512])
        mask_f = consts.tile([128, 128], dt.float32)
        nc.sync.dma_start(mask_f, Msk)
        mask = consts.tile([128, 128], ATT_DT)
        nc.vector.tensor_copy(mask, mask_f)
        ones64 = consts.tile([128, NST * 4], dt.float32)
        nc.vector.memset(ones64, 1.0)
        ones_f = consts.tile([1, 128], dt.float32)
        nc.vector.memset(ones_f, 1.0)
        ones_r = consts.tile([1, 128], dt.float32r)
        nc.vector.tensor_copy(ones_r, ones_f)

        # unnormalized attn^T and per-(head, t) softmax denominators;
        # head lh's denominators live at partition 32*(lh//2), plane lh%2
        # (single-partition engine accesses must be 32-aligned)
        attnT = apool.tile([128, HPC // 2, T], dt.float32r)
        dns = consts.tile([128, 2, T], dt.float32)

        kTs, qTs, vs = {}, {}, {}

        def proj_chunks(r):
            """Emit W DMAs; return list of closures, each emitting one
            PSUM matmul group of round r's projections."""
            chunks = []
            kT = kpool.tile([128, T], ATT_DT, name=f"kT_{r}", tag="kT")
            qT = qpool.tile([128, T], ATT_DT, name=f"qT_{r}", tag="qT")
            kTs[r], qTs[r] = kT, qT
            colK = 512 + r * 128
            wk = [wkq.tile([128, 128], dt.float32r, tag="wk", name=f"wk_{r}_{a}")
                  for a in range(CT)]
            colQ = r * 128
            wq = [wkq.tile([128, 128], dt.float32r, tag="wk", name=f"wq_{r}_{a}")
                  for a in range(CT)]
            for a in range(CT):
                nc.sync.dma_start(wk[a], Wa[a * 128:(a + 1) * 128, colK:colK + 128])
                nc.sync.dma_start(wq[a], Wa[a * 128:(a + 1) * 128, colQ:colQ + 128])
            if r % 2 == 0:
                v = vpool.tile([128, NST, 4, 65], ATT_DT, name=f"v_{r}", tag="v")
                vs[r // 2] = v
                colV = 1024 + r * 128
                wv = [wvp.tile([128, 256], dt.float32r, tag="wv", name=f"wv_{r}_{a}")
                      for a in range(CT)]
                for a in range(CT):
                    nc.sync.dma_start(wv[a],
                                      Wa[a * 128:(a + 1) * 128, colV:colV + 256])

            def kq_group(w, dst, tt):
                def emit():
                    p = ps_proj.tile([128, 512], dt.float32, tag="proj",
                                     name=f"pp_{r}_{tt}")
                    for a in range(CT):
                        nc.tensor.matmul(p, w[a], xt[:, a, tt * 512:(tt + 1) * 512],
                                         start=(a == 0), stop=(a == CT - 1))
                    nc.vector.tensor_copy(dst[:, tt * 512:(tt + 1) * 512], p)
                return emit

            def v_group(st):
                def emit():
                    p = ps_proj.tile([128, 512], dt.float32, tag="proj",
                                     name=f"pv_{r}_{st}")
                    for a in range(CT):
                        nc.tensor.matmul(p[:, 0:256],
                                         xt[:, a, st * 128:(st + 1) * 128],
                                         wv[a], start=(a == 0), stop=(a == CT - 1))
                    nc.vector.tensor_copy(v[:, st, :, 0:64], p[:, 0:256])
                    if st == NST - 1:
                        nc.vector.tensor_copy(v[:, :, :, 64:65], ones64)
                return emit

            for tt in range(NTT):
                chunks.append(kq_group(wk, kT, tt))
            for tt in range(NTT):
                chunks.append(kq_group(wq, qT, tt))
            if r % 2 == 0:
                for st in range(NST):
                    chunks.append(v_group(st))
            return chunks

        def attention_unit(r, lh_i, qt):
            kT, qT, v = kTs[r], qTs[r], vs[r // 2]
            lh = 2 * r + lh_i
            o = lh_i * 64
            pv_i = lh % 4
            t_q = qt * 512
            n_s = (t_q + 512) // 128
            pvv = ps_pv.tile([128, 512], dt.float32, tag="pv",
                             name=f"pvv_{lh}_{qt}")
            wts = {}

            def do_scores(s):
                kv0 = s * 128
                qlo = 0 if kv0 < t_q else kv0 - t_q
                psc = ps_sc.tile([128, 512], dt.float32, tag="sc",
                                 name=f"psc_{lh}_{qt}_{s}")
                nc.tensor.matmul(
                    psc[:, qlo:512],
                    kT[o:o + 64, kv0:kv0 + 128],
                    qT[o:o + 64, t_q + qlo:t_q + 512],
                    start=True, stop=True)
                wt = wei.tile([128, 512], ATT_DT, tag="wt",
                              name=f"wt_{lh}_{qt}_{s}")
                nc.scalar.activation(wt[:, qlo:512], psc[:, qlo:512],
                                     func=mybir.ActivationFunctionType.Exp,
                                     scale=float(D) ** -0.5)
                if kv0 >= t_q:
                    nc.vector.tensor_tensor(
                        wt[:, qlo:qlo + 128], wt[:, qlo:qlo + 128], mask,
                        op=mybir.AluOpType.mult)
                wts[s] = (wt, qlo)

            def do_wv(s):
                wt, qlo = wts.pop(s)
                nc.tensor.matmul(
                    pvv[0:65, qlo:512],
                    v[:, s, pv_i, :],
                    wt[:, qlo:512],
                    start=(s == 0), stop=(s == n_s - 1))

            for s in range(n_s):
                do_scores(s)
                if s >= 1:
                    do_wv(s - 1)
            do_wv(n_s - 1)

            # stash unnormalized out^T and the denominator row
            nc.vector.tensor_copy(attnT[o:o + 64, r, t_q:t_q + 512], pvv[0:64, :])
            g2 = 32 * (lh // 2)
            nc.vector.tensor_copy(dns[g2:g2 + 1, lh % 2, t_q:t_q + 512],
                                  pvv[64:65, :])

        def normalize_chunk(g, tt):
            rep = small.tile([128, 512], dt.float32, tag="rep",
                             name=f"rep_{g}_{tt}")
            for par in range(2):
                rc1 = small.tile([1, 512], dt.float32r, tag=f"rc1_{par}",
                                 name=f"rc1_{g}_{tt}_{par}")
                nc.vector.tensor_copy(
                    rc1, dns[32 * g:32 * g + 1, par, tt * 512:(tt + 1) * 512])
                prep = ps_misc.tile([64, 512], dt.float32, tag="misc",
                                    name=f"prep_{g}_{tt}_{par}")
                nc.tensor.matmul(prep, ones_r[:, 0:64], rc1,
                                 start=True, stop=True)
                nc.vector.tensor_copy(rep[64 * par:64 * par + 64, :], prep)
            nc.vector.tensor_tensor(
                attnT[:, g, tt * 512:(tt + 1) * 512],
                attnT[:, g, tt * 512:(tt + 1) * 512], rep,
                op=mybir.AluOpType.mult)

        def outproj_chunk(oc, tt8):
            po = ps_misc.tile([128, 512], dt.float32, tag="misc",
                              name=f"po_{oc}_{tt8}")
            for a in range(HPC // 2):
                nc.tensor.matmul(po, attnT[:, a, tt8 * 128:(tt8 + 1) * 128],
                                 wpts[oc][:, a, :], start=(a == 0),
                                 stop=(a == HPC // 2 - 1))
            osb = outsb.tile([128, 512], dt.float32, tag="osb",
                             name=f"osb_{oc}_{tt8}")
            nc.vector.tensor_copy(osb, po)
            nc.sync.dma_start(
                Out[tt8 * 128:(tt8 + 1) * 128, oc * 512:(oc + 1) * 512], osb)

        def tail_chunks(tt):
            """Everything that becomes ready once all heads finished q-tile
            tt: denominator reciprocal slice, normalization, out-proj."""
            # reciprocal via ACT: 1/d = exp(-ln(d)); in-place on dns slice
            sl = dns[:, :, tt * 512:(tt + 1) * 512]
            nc.scalar.activation(sl, sl, func=mybir.ActivationFunctionType.Ln)
            nc.scalar.activation(sl, sl, func=mybir.ActivationFunctionType.Exp,
                                 scale=-1.0)
            for g in range(HPC // 2):
                normalize_chunk(g, tt)
            for oc in range(2):
                for tt8 in range(4 * tt, 4 * tt + 4):
                    outproj_chunk(oc, tt8)

        # ---- software-pipelined emission ----
        wpts = {}
        for c in proj_chunks(0):
            c()
        for r in range(ROUNDS):
            if r == ROUNDS - 1:
                for oc in range(2):
                    wpt = wpp.tile([128, HPC // 2, 512], dt.float32r,
                                   name=f"wpt_{oc}", tag="wpt")
                    wpts[oc] = wpt
                    for a in range(HPC // 2):
                        nc.sync.dma_start(
                            wpt[:, a, :],
                            Wp[a * 128:(a + 1) * 128, oc * 512:(oc + 1) * 512])
            nxt = proj_chunks(r + 1) if r + 1 < ROUNDS else []
            units = [(lh_i, qt) for qt in range(NQT) for lh_i in range(2)]
            per = (len(nxt) + len(units) - 1) // len(units) if nxt else 0
            for i, (lh_i, qt) in enumerate(units):
                attention_unit(r, lh_i, qt)
                for c in nxt[i * per:(i + 1) * per]:
                    c()
                if r == ROUNDS - 1:
                    # emit each q-tile's tail one unit late so the PE has
                    # attention work while the tail's DVE chain resolves
                    if lh_i == 0 and qt >= 1:
                        tail_chunks(qt - 1)
                    elif lh_i == 1 and qt == NQT - 1:
                        tail_chunks(qt)

    nc.compile()
    return nc


def kernel(x, W_attn, W_proj, _trace=False):
    global _compiled, last_exec_time_ns
    x = np.asarray(x, dtype=np.float32)
    W_attn = np.asarray(W_attn, dtype=np.float32)
    W_proj = np.asarray(W_proj, dtype=np.float32)

    if _compiled is None:
        _compiled = _build()
    nc = _compiled

    mask_np = np.triu(np.ones((128, 128), dtype=np.float32))
    in_maps = []
    for c in range(N_CORES):
        b, hf = c // 2, c % 2
        xT_c = np.ascontiguousarray(x[b].T)
        Wa_c = np.ascontiguousarray(np.concatenate([
            W_attn[:, hf * 512:(hf + 1) * 512],                  # Q cols
            W_attn[:, C + hf * 512:C + (hf + 1) * 512],          # K cols
            W_attn[:, 2 * C + hf * 512:2 * C + (hf + 1) * 512],  # V cols
        ], axis=1))
        Wp_c = np.ascontiguousarray(W_proj[hf * 512:(hf + 1) * 512, :])
        in_maps.append({"xT": xT_c, "Wa": Wa_c, "Wp": Wp_c, "Msk": mask_np})

    res = run_bass_kernel_spmd(nc, in_maps, list(range(N_CORES)), trace=_trace)
    last_exec_time_ns = res.exec_time_ns

    out = np.empty((B, T, C), dtype=np.float32)
    for b in range(B):
        out[b] = res.results[2 * b]["outp"] + res.results[2 * b + 1]["outp"]
    return out


# revision 17
# speedup vs baseline: 1.5911x; 1.1169x over previous
"""Causal self-attention (B=4, T=2048, C=1024, H=16) on 8 Trainium2 NeuronCores.

Sharding: data-parallel on batch x tensor-parallel on heads.
  core c -> batch b = c//2, head half hf = c%2 (8 heads of 16).
Each core: QKV projection for its heads (column-parallel W_attn),
flash-style causal attention fully on-chip, row-parallel W_proj
producing a partial [T, C] output; pairs of partials are summed at
unshard time. All 8 cores run an identical program (SPMD) on
different data.

Projections run in fp32r (TRN2 PE mode: fp32 storage, 11-bit-mantissa
products, bf16-rate throughput at moving-dim >= 256); attention
matmuls run in bf16.

Structure: emission is software-pipelined - projection matmul groups
of round r+1 are emitted between attention units of round r so the
PE instruction stream stays dense (keeps the HAM clock gate at full
rate); inside a unit, scores matmuls run one kv-tile ahead of the
weiV matmuls to hide the exp latency. Softmax normalization is
deferred: unnormalized outputs and denominators accumulate, then one
batched reciprocal + per-tile scale pass runs before the output
projection.

Layout: host feeds x^T per batch ([C, T]) so QKV projections need no
on-device transposes; attention runs in "scores-transposed" [kv, q]
layout; the softmax denominator comes free from a ones-column
appended to V; denominator replication uses small K=1/K=2 PE matmuls.
"""
import numpy as np
import ml_dtypes
from contextlib import ExitStack

import concourse.bacc as bacc
import concourse.tile as tile
from concourse import mybir
from concourse.bass_utils import run_bass_kernel_spmd

dt = mybir.dt

B, T, C, H = 4, 2048, 1024, 16
D = C // H          # 64
N_CORES = 8
HPC = H // 2        # heads per core = 8
ROUNDS = 4          # 2 heads per round
CT = C // 128       # 8 c-tiles (contraction for projections)
NQT = T // 512      # 4 q-tiles per head
NTT = T // 512      # 4 t-tiles for K/Q projection
NST = T // 128      # 16 s-tiles for V projection

ATT_DT = dt.bfloat16    # dtype for attention matmuls (kT/qT/v/wei)
PROJ_DT = dt.bfloat16   # dtype for projection matmuls (xt/Wa/Wp/attnT)
_NP_PROJ = "bfloat16" if PROJ_DT == dt.bfloat16 else "float32"

_compiled = None
last_exec_time_ns = None


def _build():
    nc = bacc.Bacc("TRN2", target_bir_lowering=False, debug=False,
                   num_devices=N_CORES)

    xT = nc.dram_tensor("xT", [C, T], PROJ_DT, kind="ExternalInput").ap()
    Wa = nc.dram_tensor("Wa", [C, 3 * 512], PROJ_DT, kind="ExternalInput").ap()
    Wp = nc.dram_tensor("Wp", [512, C], PROJ_DT, kind="ExternalInput").ap()
    Msk = nc.dram_tensor("Msk", [128, 128], dt.float32, kind="ExternalInput").ap()
    Out = nc.dram_tensor("outp", [T, C], dt.float32, kind="ExternalOutput").ap()

    with tile.TileContext(nc) as tc, ExitStack() as ctx:
        consts = ctx.enter_context(tc.tile_pool(name="consts", bufs=1))
        kpool = ctx.enter_context(tc.tile_pool(name="kpool", bufs=2))
        qpool = ctx.enter_context(tc.tile_pool(name="qpool", bufs=2))
        vpool = ctx.enter_context(tc.tile_pool(name="vpool", bufs=2))
        apool = ctx.enter_context(tc.tile_pool(name="apool", bufs=1))
        wkq = ctx.enter_context(tc.tile_pool(name="wkq", bufs=16))
        wvp = ctx.enter_context(tc.tile_pool(name="wvp", bufs=8))
        wpp = ctx.enter_context(tc.tile_pool(name="wpp", bufs=2))
        wei = ctx.enter_context(tc.tile_pool(name="wei", bufs=6))
        small = ctx.enter_context(tc.tile_pool(name="small", bufs=3))
        outsb = ctx.enter_context(tc.tile_pool(name="outsb", bufs=2))
        ps_proj = ctx.enter_context(tc.tile_pool(name="ps_proj", bufs=1, space="PSUM"))
        ps_sc = ctx.enter_context(tc.tile_pool(name="ps_sc", bufs=3, space="PSUM"))
        ps_pv = ctx.enter_context(tc.tile_pool(name="ps_pv", bufs=2, space="PSUM"))
        ps_misc = ctx.enter_context(tc.tile_pool(name="ps_misc", bufs=2, space="PSUM"))

        # ---- constants & x^T ----
        xt = consts.tile([128, CT, T], PROJ_DT)

        def emit_xt_dmas():
            for tq in range(4):
                for a in range(CT):
                    nc.gpsimd.dma_start(
                        xt[:, a, tq * 512:(tq + 1) * 512],
                        xT[a * 128:(a + 1) * 128, tq * 512:(tq + 1) * 512])
        mask_f = consts.tile([128, 128], dt.float32)
        nc.sync.dma_start(mask_f, Msk)
        mask = consts.tile([128, 128], ATT_DT)
        nc.vector.tensor_copy(mask, mask_f)
        ones64 = consts.tile([128, NST * 4], dt.float32)
        nc.vector.memset(ones64, 1.0)
        ones_f = consts.tile([1, 128], dt.float32)
        nc.vector.memset(ones_f, 1.0)
        ones_r = consts.tile([1, 128], dt.float32r)
        nc.vector.tensor_copy(ones_r, ones_f)

        # unnormalized attn^T and per-(head, t) softmax denominators;
        # head lh's denominators live at partition 32*(lh//2), plane lh%2
        # (single-partition engine accesses must be 32-aligned)
        attnT = apool.tile([128, HPC // 2, T], PROJ_DT)
        dns = consts.tile([128, 2, T], dt.float32)

        kTs, qTs, vs = {}, {}, {}

        def proj_chunks(r):
            """Emit W DMAs; return list of closures, each emitting one
            PSUM matmul group of round r's projections."""
            chunks = []
            kT = kpool.tile([128, T], ATT_DT, name=f"kT_{r}", tag="kT")
            qT = qpool.tile([128, T], ATT_DT, name=f"qT_{r}", tag="qT")
            kTs[r], qTs[r] = kT, qT
            colK = 512 + r * 128
            wk = [wkq.tile([128, 128], PROJ_DT, tag="wk", name=f"wk_{r}_{a}")
                  for a in range(CT)]
            colQ = r * 128
            wq = [wkq.tile([128, 128], PROJ_DT, tag="wk", name=f"wq_{r}_{a}")
                  for a in range(CT)]
            for a in range(CT):
                nc.sync.dma_start(wk[a], Wa[a * 128:(a + 1) * 128, colK:colK + 128])
                nc.sync.dma_start(wq[a], Wa[a * 128:(a + 1) * 128, colQ:colQ + 128])
            if r % 2 == 0:
                v = vpool.tile([128, NST, 4, 65], ATT_DT, name=f"v_{r}", tag="v")
                vs[r // 2] = v
                colV = 1024 + r * 128
                wv = [wvp.tile([128, 256], PROJ_DT, tag="wv", name=f"wv_{r}_{a}")
                      for a in range(CT)]
                for a in range(CT):
                    nc.sync.dma_start(wv[a],
                                      Wa[a * 128:(a + 1) * 128, colV:colV + 256])

            def kq_group(w, dst, tt):
                def emit():
                    p = ps_proj.tile([128, 512], dt.float32, tag="proj",
                                     name=f"pp_{r}_{tt}")
                    for a in range(CT):
                        nc.tensor.matmul(p, w[a], xt[:, a, tt * 512:(tt + 1) * 512],
                                         start=(a == 0), stop=(a == CT - 1))
                    nc.vector.tensor_copy(dst[:, tt * 512:(tt + 1) * 512], p)
                return emit

            def v_group(st):
                def emit():
                    p = ps_proj.tile([128, 512], dt.float32, tag="proj",
                                     name=f"pv_{r}_{st}")
                    for a in range(CT):
                        nc.tensor.matmul(p[:, 0:256],
                                         xt[:, a, st * 128:(st + 1) * 128],
                                         wv[a], start=(a == 0), stop=(a == CT - 1))
                    nc.vector.tensor_copy(v[:, st, :, 0:64], p[:, 0:256])
                    if st == NST - 1:
                        nc.vector.tensor_copy(v[:, :, :, 64:65], ones64)
                return emit

            for tt in range(NTT):
                chunks.append(kq_group(wk, kT, tt))
            for tt in range(NTT):
                chunks.append(kq_group(wq, qT, tt))
            if r % 2 == 0:
                for st in range(NST):
                    chunks.append(v_group(st))
            return chunks

        def attention_unit(r, lh_i, qt):
            kT, qT, v = kTs[r], qTs[r], vs[r // 2]
            lh = 2 * r + lh_i
            o = lh_i * 64
            pv_i = lh % 4
            t_q = qt * 512
            n_s = (t_q + 512) // 128
            pvv = ps_pv.tile([128, 512], dt.float32, tag="pv",
                             name=f"pvv_{lh}_{qt}")
            wts = {}

            def do_scores(s):
                kv0 = s * 128
                qlo = 0 if kv0 < t_q else kv0 - t_q
                psc = ps_sc.tile([128, 512], dt.float32, tag="sc",
                                 name=f"psc_{lh}_{qt}_{s}")
                nc.tensor.matmul(
                    psc[:, qlo:512],
                    kT[o:o + 64, kv0:kv0 + 128],
                    qT[o:o + 64, t_q + qlo:t_q + 512],
                    start=True, stop=True)
                wt = wei.tile([128, 512], ATT_DT, tag="wt",
                              name=f"wt_{lh}_{qt}_{s}")
                nc.scalar.activation(wt[:, qlo:512], psc[:, qlo:512],
                                     func=mybir.ActivationFunctionType.Exp,
                                     scale=float(D) ** -0.5)
                if kv0 >= t_q:
                    nc.vector.tensor_tensor(
                        wt[:, qlo:qlo + 128], wt[:, qlo:qlo + 128], mask,
                        op=mybir.AluOpType.mult)
                wts[s] = (wt, qlo)

            def do_wv(s):
                wt, qlo = wts.pop(s)
                nc.tensor.matmul(
                    pvv[0:65, qlo:512],
                    v[:, s, pv_i, :],
                    wt[:, qlo:512],
                    start=(s == 0), stop=(s == n_s - 1))

            for s in range(n_s):
                do_scores(s)
                if s >= 1:
                    do_wv(s - 1)
            do_wv(n_s - 1)

            # stash unnormalized out^T and the denominator row
            nc.vector.tensor_copy(attnT[o:o + 64, r, t_q:t_q + 512], pvv[0:64, :])
            g2 = 32 * (lh // 2)
            nc.vector.tensor_copy(dns[g2:g2 + 1, lh % 2, t_q:t_q + 512],
                                  pvv[64:65, :])

        def normalize_chunk(g, tt):
            rep = small.tile([128, 512], dt.float32, tag="rep",
                             name=f"rep_{g}_{tt}")
            for par in range(2):
                rc1 = small.tile([1, 512], dt.float32r, tag=f"rc1_{par}",
                                 name=f"rc1_{g}_{tt}_{par}")
                nc.vector.tensor_copy(
                    rc1, dns[32 * g:32 * g + 1, par, tt * 512:(tt + 1) * 512])
                prep = ps_misc.tile([64, 512], dt.float32, tag="misc",
                                    name=f"prep_{g}_{tt}_{par}")
                nc.tensor.matmul(prep, ones_r[:, 0:64], rc1,
                                 start=True, stop=True)
                nc.vector.tensor_copy(rep[64 * par:64 * par + 64, :], prep)
            nc.vector.tensor_tensor(
                attnT[:, g, tt * 512:(tt + 1) * 512],
                attnT[:, g, tt * 512:(tt + 1) * 512], rep,
                op=mybir.AluOpType.mult)

        def outproj_chunk(oc, tt8):
            po = ps_misc.tile([128, 512], dt.float32, tag="misc",
                              name=f"po_{oc}_{tt8}")
            for a in range(HPC // 2):
                nc.tensor.matmul(po, attnT[:, a, tt8 * 128:(tt8 + 1) * 128],
                                 wpts[oc][:, a, :], start=(a == 0),
                                 stop=(a == HPC // 2 - 1))
            osb = outsb.tile([128, 512], dt.float32, tag="osb",
                             name=f"osb_{oc}_{tt8}")
            nc.vector.tensor_copy(osb, po)
            nc.sync.dma_start(
                Out[tt8 * 128:(tt8 + 1) * 128, oc * 512:(oc + 1) * 512], osb)

        def tail_chunks(tt):
            """Everything that becomes ready once all heads finished q-tile
            tt: denominator reciprocal slice, normalization, out-proj."""
            # reciprocal via ACT: 1/d = exp(-ln(d)); in-place on dns slice
            sl = dns[:, :, tt * 512:(tt + 1) * 512]
            nc.scalar.activation(sl, sl, func=mybir.ActivationFunctionType.Ln)
            nc.scalar.activation(sl, sl, func=mybir.ActivationFunctionType.Exp,
                                 scale=-1.0)
            for g in range(HPC // 2):
                normalize_chunk(g, tt)
            for oc in range(2):
                for tt8 in range(4 * tt, 4 * tt + 4):
                    outproj_chunk(oc, tt8)

        # ---- software-pipelined emission ----
        wpts = {}
        chunks0 = proj_chunks(0)   # emits round-0 weight DMAs first
        emit_xt_dmas()
        for c in chunks0:
            c()
        for r in range(ROUNDS):
            if r == ROUNDS - 1:
                for oc in range(2):
                    wpt = wpp.tile([128, HPC // 2, 512], PROJ_DT,
                                   name=f"wpt_{oc}", tag="wpt")
                    wpts[oc] = wpt
                    for a in range(HPC // 2):
                        nc.sync.dma_start(
                            wpt[:, a, :],
                            Wp[a * 128:(a + 1) * 128, oc * 512:(oc + 1) * 512])
            nxt = proj_chunks(r + 1) if r + 1 < ROUNDS else []
            units = [(lh_i, qt) for qt in range(NQT) for lh_i in range(2)]
            per = (len(nxt) + len(units) - 1) // len(units) if nxt else 0
            for i, (lh_i, qt) in enumerate(units):
                attention_unit(r, lh_i, qt)
                for c in nxt[i * per:(i + 1) * per]:
                    c()
                if r == ROUNDS - 1:
                    # emit each q-tile's tail one unit late so the PE has
                    # attention work while the tail's DVE chain resolves
                    if lh_i == 0 and qt >= 1:
                        tail_chunks(qt - 1)
                    elif lh_i == 1 and qt == NQT - 1:
                        tail_chunks(qt)

    nc.compile()
    return nc


def kernel(x, W_attn, W_proj, _trace=False):
    global _compiled, last_exec_time_ns
    x = np.asarray(x, dtype=np.float32)
    W_attn = np.asarray(W_attn, dtype=np.float32)
    W_proj = np.asarray(W_proj, dtype=np.float32)

    if _compiled is None:
        _compiled = _build()
    nc = _compiled

    mask_np = np.triu(np.ones((128, 128), dtype=np.float32))
    in_maps = []
    for c in range(N_CORES):
        b, hf = c // 2, c % 2
        xT_c = np.ascontiguousarray(x[b].T).astype(_NP_PROJ)
        Wa_c = np.ascontiguousarray(np.concatenate([
            W_attn[:, hf * 512:(hf + 1) * 512],                  # Q cols
            W_attn[:, C + hf * 512:C + (hf + 1) * 512],          # K cols
            W_attn[:, 2 * C + hf * 512:2 * C + (hf + 1) * 512],  # V cols
        ], axis=1)).astype(_NP_PROJ)
        Wp_c = np.ascontiguousarray(W_proj[hf * 512:(hf + 1) * 512, :]).astype(_NP_PROJ)
        in_maps.append({"xT": xT_c, "Wa": Wa_c, "Wp": Wp_c, "Msk": mask_np})

    res = run_bass_kernel_spmd(nc, in_maps, list(range(N_CORES)), trace=_trace)
    last_exec_time_ns = res.exec_time_ns

    out = np.empty((B, T, C), dtype=np.float32)
    for b in range(B):
        out[b] = res.results[2 * b]["outp"] + res.results[2 * b + 1]["outp"]
    return out
